# revision 23
# baseline (speedup 1.0000x reference)
"""Detection-loss kernel for Trainium2 (8 NeuronCores, data-parallel over batch).

Reference computes: scatter 64 targets/image into a [B,C,H,W] map + mask,
then masked SmoothL1(preds, map).sum() / num_objects.

Key observation: the mask is nonzero at <= B*T positions, so the loss only
depends on preds at those positions.  Instead of streaming the 143MB preds
tensor, each core *gathers* preds at its images' (gy,gx) cells via indirect
DMA (1792 elements/core), resolves duplicate-cell collisions with
last-writer-wins (matching jax scatter semantics), and reduces two partial
scalars.  Host combines the 8 partial pairs.

Sharding layout per core (4 images, 2 groups of 128 targets on partitions):
  partition p in [0,128), group g in {0,1}:
    image j = g*2 + p//64 (local), target t = p%64, channel c in [0,7)
  preds are host-relayouted channels-last ([b,y,x,c]) so one indirect-DMA
  descriptor per target moves all 7 channels (28B contiguous).
  flat gather offset = (gy*W + gx)*C + j*C*H*W, gy/gx = floor(coord * 5.0).
"""

import numpy as np

B, C, H, W = 32, 7, 400, 400
T = 64
NCORES = 8
BLOC = B // NCORES          # 4 images per core
HW = H * W                  # 160000
CHW = C * HW                # 1120000
NELEM = BLOC * CHW          # 4480000 elements per core
NG = BLOC * T // 128        # 2 groups of 128 targets
P = 128
GC = NG * C                 # 14 value columns
BIGM = float(2**25)         # collision-mask offset (kills eq below diagonal)

_cached = {}
TRACE = False  # set True (e.g. from test.py) to capture an NTFF profile


def _build_nc():
    import concourse.bacc as bacc
    import concourse.bass as bass
    import concourse.tile as tile
    import concourse.mybir as mybir

    f32 = mybir.dt.float32
    i32 = mybir.dt.int32
    OP = mybir.AluOpType
    AX = mybir.AxisListType

    nc = bacc.Bacc(
        "TRN2",
        target_bir_lowering=False,
        debug=False,
        enable_asserts=False,
        num_devices=NCORES,
    )

    preds_flat = nc.dram_tensor("preds_flat", [NELEM, 1], f32, kind="ExternalInput")
    # aux1: [t01 (4) | jbase (2)] — the small operands the coord chain needs
    aux1 = nc.dram_tensor("aux1", [P, 6], f32, kind="ExternalInput")
    # aux2: [tvals (14) | utm (128) | id128 (128)]
    aux2 = nc.dram_tensor("aux2", [P, GC + 2 * P], f32, kind="ExternalInput")
    out_d = nc.dram_tensor("out", [2, 1], f32, kind="ExternalOutput")

    with tile.TileContext(nc) as tc:
        with (
            tc.tile_pool(name="sbuf", bufs=1) as sb,
            tc.tile_pool(name="psum", bufs=1, space="PSUM") as pp,
        ):
            x1 = sb.tile([P, 6], f32)
            nc.sync.dma_start(x1[:], aux1[:, :])
            x2 = sb.tile([P, GC + 2 * P], f32)
            nc.sync.dma_start(x2[:], aux2[:, :])
            tv = x2[:, 0:GC]
            utm = x2[:, GC : GC + P]
            idt = x2[:, GC + P : GC + 2 * P]

            # grid coords: floor(coord*5) via int32 round-trip (any rounding
            # mode) corrected where the round-trip exceeded the input
            a = sb.tile([P, 2 * NG], f32)
            nc.vector.tensor_scalar_mul(a[:], x1[:, 0 : 2 * NG], 5.0)
            ci = sb.tile([P, 2 * NG], i32)
            nc.vector.tensor_copy(ci[:], a[:])
            cf = sb.tile([P, 2 * NG], f32)
            nc.vector.tensor_copy(cf[:], ci[:])
            corr = sb.tile([P, 2 * NG], f32)
            nc.vector.tensor_tensor(corr[:], cf[:], a[:], OP.is_gt)
            fl = sb.tile([P, 2 * NG], f32)
            nc.vector.tensor_sub(fl[:], cf[:], corr[:])
            # npos = gy*W + gx ; offs = npos*C + j*C*H*W   (exact ints < 2^23)
            npos = sb.tile([P, NG], f32)
            nc.vector.scalar_tensor_tensor(
                npos[:], fl[:, NG : 2 * NG], float(W), fl[:, 0:NG], OP.mult, OP.add
            )
            offs_f = sb.tile([P, NG], f32)
            nc.vector.scalar_tensor_tensor(
                offs_f[:], npos[:], float(C), x1[:, 4:6], OP.mult, OP.add
            )
            offs_i = sb.tile([P, NG], i32)
            nc.vector.tensor_copy(offs_i[:], offs_f[:])

            # gather: one 28B descriptor per target (channels-last layout)
            gat = sb.tile([P, GC], f32)
            for g in range(NG):
                nc.gpsimd.indirect_dma_start(
                    out=gat[:, g * C : (g + 1) * C],
                    out_offset=None,
                    in_=preds_flat[:, :],
                    in_offset=bass.IndirectOffsetOnAxis(
                        ap=offs_i[:, g : g + 1], axis=0
                    ),
                )

            # last-writer-wins winner mask per group (overlaps the gather):
            # pT[p,q] = pos[q] (PE transpose, bit-exact); +BIGM on/below the
            # diagonal makes eq impossible there, so a row-max of equality
            # flags collisions with a LATER target.
            win2 = sb.tile([P, NG], f32)
            for g in range(NG):
                posb = offs_f[:, g : g + 1].to_broadcast([P, P])
                pT_ps = pp.tile([P, P], f32, tag=f"tps{g}")
                nc.tensor.transpose(pT_ps[:], posb, idt[:])
                pTm = sb.tile([P, P], f32, tag=f"pTm{g}")
                nc.vector.tensor_add(pTm[:], pT_ps[:], utm[:])
                eq = sb.tile([P, P], f32, tag=f"eq{g}")
                nc.vector.tensor_tensor(eq[:], posb, pTm[:], OP.is_equal)
                coll = sb.tile([P, 1], f32, tag=f"coll{g}")
                nc.vector.reduce_max(coll[:], eq[:], axis=AX.X)
                nc.vector.tensor_scalar(
                    win2[:, g : g + 1], coll[:], -1.0, 1.0, OP.mult, OP.add
                )

            # pre-gather (off the gather critical path): win-sum column and
            # the 0.5*winner loss factor
            rhs = sb.tile([P, 2], f32)
            nc.vector.tensor_add(rhs[:, 1:2], win2[:, 0:1], win2[:, 1:2])
            halfwin = sb.tile([P, NG], f32)
            nc.vector.tensor_scalar_mul(halfwin[:], win2[:], 0.5)

            # smoothl1(d)*win = (0.5*win*min(|d|,1)) * (|d| + relu(|d|-1));
            # winner folded into the min factor so the full 14-wide row sum
            # is the loss partial directly
            d = sb.tile([P, GC], f32)
            nc.vector.tensor_sub(d[:], gat[:], tv[:])
            ad = sb.tile([P, GC], f32)
            nc.vector.scalar_tensor_tensor(ad[:], d[:], -1.0, d[:], OP.mult, OP.max)
            mn = sb.tile([P, GC], f32)
            nc.vector.tensor_scalar_min(mn[:], ad[:], 1.0)
            mw = sb.tile([P, GC], f32)
            for g in range(NG):
                nc.vector.tensor_scalar_mul(
                    mw[:, g * C : (g + 1) * C],
                    mn[:, g * C : (g + 1) * C],
                    halfwin[:, g : g + 1],
                )
            r = sb.tile([P, GC], f32)
            nc.vector.tensor_scalar(r[:], ad[:], 1.0, 0.0, OP.subtract, OP.max)
            s = sb.tile([P, GC], f32)
            nc.vector.tensor_add(s[:], ad[:], r[:])
            le = sb.tile([P, GC], f32)
            nc.vector.tensor_mul(le[:], mw[:], s[:])
            nc.vector.reduce_sum(rhs[:, 0:1], le[:], axis=AX.X)

            # exact partition reduction: PE transpose (bit-exact move) then
            # DVE reduce straight out of PSUM
            tps = pp.tile([2, P], f32, tag="tfin")
            nc.tensor.transpose(tps[:], rhs[:], idt[:])
            red = sb.tile([2, 1], f32)
            nc.vector.reduce_sum(red[:], tps[:], axis=AX.X)
            nc.sync.dma_start(out_d[:, :], red[:])

    nc.compile()
    return nc


def _get_nc():
    if "nc" not in _cached:
        _cached["nc"] = _build_nc()
    return _cached["nc"]


def _make_in_maps(preds, targets):
    jj = (np.arange(P) // 64)[:, None]
    gg = np.arange(NG)[None, :]
    jbase = ((gg * 2 + jj) * CHW).astype(np.float32)
    rr = np.arange(P)
    utm = np.where(rr[None, :] > rr[:, None], 0.0, BIGM).astype(np.float32)
    id128 = np.eye(P, dtype=np.float32)

    # channels-last relayout so each target's 7 channels are one contiguous
    # 28B indirect-DMA row
    preds_t = np.ascontiguousarray(preds.transpose(0, 2, 3, 1))

    in_maps = []
    for k in range(NCORES):
        pshard = preds_t[k * BLOC : (k + 1) * BLOC].reshape(NELEM, 1)
        tshard = targets[k * BLOC : (k + 1) * BLOC]  # [4, 64, 7]
        # tvals[p, g*7+c] = tshard[g*2 + p//64, p%64, c]
        tvals = tshard.reshape(NG, 2, T, C).transpose(1, 2, 0, 3).reshape(P, GC)
        # t01 cols: [x_g0, x_g1, y_g0, y_g1]
        t01 = np.stack(
            [tvals[:, 0], tvals[:, C], tvals[:, 1], tvals[:, C + 1]], axis=1
        )
        aux1 = np.ascontiguousarray(np.hstack([t01, jbase]).astype(np.float32))
        aux2 = np.ascontiguousarray(
            np.hstack([tvals, utm, id128]).astype(np.float32)
        )
        in_maps.append({"preds_flat": pshard, "aux1": aux1, "aux2": aux2})
    return in_maps


def kernel(preds, targets):
    from concourse.bass_utils import run_bass_kernel_spmd

    preds = np.ascontiguousarray(np.asarray(preds), dtype=np.float32)
    targets = np.ascontiguousarray(np.asarray(targets), dtype=np.float32)
    assert preds.shape == (B, C, H, W) and targets.shape == (B, T, C)

    nc = _get_nc()
    in_maps = _make_in_maps(preds, targets)
    res = run_bass_kernel_spmd(nc, in_maps, list(range(NCORES)), trace=TRACE)
    _cached["last_results"] = res

    lsum = np.float32(0.0)
    nsum = np.float32(0.0)
    for k in range(NCORES):
        part = res.results[k]["out"].reshape(2)
        lsum = np.float32(lsum + np.float32(part[0]))
        nsum = np.float32(nsum + np.float32(part[1]))
    loss = np.float32(lsum / np.float32(nsum + np.float32(1e-6)))
    return loss, nsum
